# revision 1
# baseline (speedup 1.0000x reference)
"""CBAM kernel for Trainium2, 8-way batch-parallel SPMD — v2.

out = x^2 * (att_c[b,c] + sigmoid(conv(spatial_stats))[b,l]) per the CBAM
reference (out = x*ca + x*sa with ca = x*att_c, sa = x*sigmoid(conv)).

v2 layout: per core 4 batches; batch x[4096, 256] lives in SBUF as one
[128, 8192] bf16 tensor with partition p = l // 32, free col = 256*(l % 32)
+ c.  Per-partition DRAM chunks are 32 KB contiguous, so each batch loads
with ONE SWDGE cast-DMA (fp32 -> bf16) at near line rate, and stores with
one HWDGE fp32 DMA from an fp32 out buffer.

Engine split per batch:
  PE   : channel-sum (1/L ones-column matmuls), MLP, stats transposes,
         conv over L as banded-Toeplitz matmuls on the transposed stats
  ACT  : squares (into the fp32 out buffer), sigmoids, relu, PSUM copies
  DVE  : spatial sum/max fold trees (bf16 2x), channel-max fold tree,
         chan-max cross-partition machinery, part of the final
         (att + sig) * x^2 scalar_tensor_tensor ops
  POOL : input cast-DMAs (SWDGE), the other part of the final stt ops
"""

import numpy as np
from contextlib import ExitStack

import ml_dtypes

import concourse.bacc as bacc
import concourse.bass as bass
import concourse.tile as tile
import concourse.mybir as mybir
from concourse.bass_utils import run_bass_kernel_spmd

AF = mybir.ActivationFunctionType
ALU = mybir.AluOpType
AX = mybir.AxisListType
FP32 = mybir.dt.float32
BF16 = mybir.dt.bfloat16

N_CORES = 8
B_FULL = 32
NB = B_FULL // N_CORES  # batches per core = 4
L = 4096
C = 256
HID = 16
HB = HID + 1
P = 128
R = 32  # L-rows per partition (l = 32*p + r)

# Final slices with r % 8 >= POOL_R_CUT run on gpsimd (Pool), rest on DVE.
POOL_R_CUT = 5

_CACHE: dict = {}


def _fold_tree(nc, pool, src_ap, nr, w0, dtype_hi, out_tile, name,
               op, f32_from):
    """Binary fold of [128, (nr, w0)] view down to [128, nr] into out_tile.

    Levels with width >= f32_from stay in dtype_hi (bf16, 2x DVE rate);
    below that, intermediates are fp32.  src_ap is the [P, nr*w0] AP.
    """
    cur = src_ap
    w = w0
    lvl = 0
    while w > 1:
        hw = w // 2
        if hw == 1:
            out = out_tile
            odt = out_tile.tensor.dtype if hasattr(out_tile, "tensor") else FP32
        else:
            dt = dtype_hi if hw >= f32_from else FP32
            out = pool.tile([P, nr * hw], dt, tag=f"{name}{lvl}",
                            name=f"{name}{lvl}")[:]
        cv = cur.rearrange("p (r c) -> p r c", c=w)
        ov = out.rearrange("p (r c) -> p r c", c=hw)
        nc.vector.tensor_tensor(ov, cv[:, :, 0:hw], cv[:, :, hw:w], op)
        cur = out
        w = hw
        lvl += 1


def _build_body(ctx: ExitStack, tc, out_d, x_d, w1_d, b1_d, w2b_d, ca_d,
                cm_d, ones_d, id_d, rc_d):
    nc = tc.nc

    const = ctx.enter_context(tc.tile_pool(name="const", bufs=1))
    xpool = ctx.enter_context(tc.tile_pool(name="x", bufs=1))
    opool = ctx.enter_context(tc.tile_pool(name="outb", bufs=1))
    fpool = ctx.enter_context(tc.tile_pool(name="fold", bufs=1))
    tpool = ctx.enter_context(tc.tile_pool(name="ptmp", bufs=2))
    spool = ctx.enter_context(tc.tile_pool(name="stats", bufs=2))
    apool = ctx.enter_context(tc.tile_pool(name="att", bufs=2))
    pacc = ctx.enter_context(tc.tile_pool(name="pacc", bufs=2, space="PSUM"))
    pwork = ctx.enter_context(tc.tile_pool(name="pwork", bufs=4, space="PSUM"))

    w1 = const.tile([P, 2 * HB], FP32)
    nc.sync.dma_start(w1[:], w1_d[:])
    b1 = const.tile([HB, 1], FP32)
    nc.sync.dma_start(b1[:], b1_d[:])
    w2b = const.tile([HB, C], FP32)
    nc.sync.dma_start(w2b[:], w2b_d[:])
    convA = const.tile([R, 3 * R], FP32)
    nc.sync.dma_start(convA[:], ca_d[:])
    convM = const.tile([R, 3 * R], FP32)
    nc.sync.dma_start(convM[:], cm_d[:])
    ones = const.tile([HB, P], FP32)
    nc.sync.dma_start(ones[:], ones_d[:])
    ident = const.tile([P, P], FP32)
    nc.sync.dma_start(ident[:], id_d[:])
    redcol = const.tile([P, 1], BF16)
    nc.sync.dma_start(redcol[:], rc_d[:])

    # ---- prefetch all four batches (SWDGE cast fp32 -> bf16) ----
    xb = []
    for b in range(NB):
        xt = xpool.tile([P, R * C], BF16, tag=f"x{b}", name=f"x{b}")
        nc.gpsimd.dma_start(
            xt[:], x_d[b, :, :].rearrange("(p r) c -> p (r c)", p=P))
        xb.append(xt)

    def compute_stats(b):
        """Squares into out buffer; stats trees; att & sig."""
        x = xb[b][:]
        ob = opool.tile([P, R * C], FP32, tag=f"ob{b % 2}", name=f"ob{b % 2}")

        # squares (ACT) straight into the fp32 out buffer
        half = R * C // 2
        nc.scalar.activation(ob[:, 0:half], x[:, 0:half], AF.Square)
        nc.scalar.activation(ob[:, half:2 * half], x[:, half:2 * half],
                             AF.Square)

        # channel sum over l (PE): lhsT = 1/L column, accumulate 32 r-slices
        pcs = pacc.tile([1, C], FP32, tag="pcs")
        for r in range(R):
            nc.tensor.matmul(pcs[:], redcol[:], x[:, C * r:C * (r + 1)],
                             start=(r == 0), stop=(r == R - 1),
                             skip_group_check=True)

        # spatial sum/max over c per (p, r): binary fold trees (DVE)
        sum_s = spool.tile([P, R], FP32, tag="sum_s")
        with nc.allow_low_precision("bf16 upper fold levels; tol 2e-2"):
            _fold_tree(nc, fpool, x, R, C, BF16, sum_s[:], "sa",
                       ALU.add, f32_from=64)
        max_s = spool.tile([P, R], FP32, tag="max_s")
        _fold_tree(nc, fpool, x, R, C, BF16, max_s[:], "sm",
                   ALU.max, f32_from=2)

        # channel max over l: fold r within partitions, then cross-partition
        mb = fpool.tile([P, R * C // 2], BF16, tag="mb", name="mb")
        nc.vector.tensor_max(mb[:], x[:, 0:R * C // 2], x[:, R * C // 2:])
        w = R * C // 4
        while w > C:
            nc.vector.tensor_max(mb[:, 0:w], mb[:, 0:w], mb[:, w:2 * w])
            w //= 2
        mbf = spool.tile([P, C], FP32, tag="mbf")
        nc.vector.tensor_max(mbf[:], mb[:, 0:C], mb[:, C:2 * C])

        # cross-partition chan-max: 32x32 block transpose, in-block reduce,
        # quadrant gather (scalar-queue DMAs), fold, scatter into stats_cm
        bt = spool.tile([P, C], FP32, tag="bt")
        nc.vector.transpose(bt[:], mbf[:])
        red = spool.tile([P, 8], FP32, tag="red")
        nc.vector.tensor_reduce(red[:],
                                bt[:].rearrange("p (bj s) -> p bj s", s=32),
                                axis=AX.X, op=ALU.max)
        cm32 = spool.tile([32, 32], FP32, tag="cm32")
        for a in range(4):
            nc.scalar.dma_start(cm32[:, 8 * a:8 * (a + 1)],
                                red[32 * a:32 * (a + 1), :])
        cmf = spool.tile([32, 8], FP32, tag="cmf")
        nc.vector.tensor_reduce(cmf[:],
                                cm32[:].rearrange("r (a bj) -> r bj a", a=4),
                                axis=AX.X, op=ALU.max)

        stats_cm = spool.tile([P, 4], FP32, tag="stats_cm")
        avg_row = spool.tile([1, C], FP32, tag="avg_row")
        nc.scalar.activation(avg_row[:], pcs[:], AF.Copy)
        for h in range(2):
            nc.scalar.dma_start(stats_cm[:, 2 * h:2 * h + 1],
                                avg_row[0:1, P * h:P * (h + 1)])
        for bj in range(8):
            q = 32 * (bj % 4)
            nc.scalar.dma_start(stats_cm[q:q + 32, 2 * (bj // 4) + 1:
                                         2 * (bj // 4) + 2],
                                cmf[:, bj:bj + 1])

        # shared MLP -> att [128, 256] f32 broadcast over partitions.
        # Row HID carries a constant 1 so w2b's b2 row contributes 2*b2.
        ph = pwork.tile([HB, 2], FP32, tag="pwork")
        nc.tensor.matmul(ph[:], w1[:, 0:HB], stats_cm[:, 0:2],
                         start=True, stop=False, skip_group_check=True)
        nc.tensor.matmul(ph[:], w1[:, HB:2 * HB], stats_cm[:, 2:4],
                         start=False, stop=True, skip_group_check=True)
        hsb = spool.tile([HB, 2], FP32, tag="hsb")
        nc.scalar.activation(hsb[:], ph[:], AF.Relu, bias=b1[:])
        h2 = spool.tile([HB, 1], FP32, tag="h2")
        nc.vector.tensor_add(h2[:], hsb[:, 0:1], hsb[:, 1:2])
        h2r = spool.tile([HB, P], FP32, tag="h2r")
        nc.vector.tensor_scalar_mul(h2r[:], ones[:], h2[:])
        po = pwork.tile([P, C], FP32, tag="pwork")
        nc.tensor.matmul(po[:], h2r[:], w2b[:], start=True, stop=True,
                         skip_group_check=True)
        att = apool.tile([P, C], FP32, tag="att")
        nc.scalar.activation(att[:], po[:], AF.Sigmoid)

        # conv over l: transpose stats to [32, 128] (l = 32*col + row),
        # banded-Toeplitz matmuls over the 32-row blocks with corner terms
        # into adjacent columns, sigmoid, transpose back.
        pta = pwork.tile([R, P], FP32, tag="pwork")
        nc.tensor.transpose(pta[:], sum_s[:], ident[:])
        stA = spool.tile([R, P], FP32, tag="stA")
        nc.scalar.activation(stA[:], pta[:], AF.Copy)
        ptm = pwork.tile([R, P], FP32, tag="pwork")
        nc.tensor.transpose(ptm[:], max_s[:], ident[:])
        stM = spool.tile([R, P], FP32, tag="stM")
        nc.scalar.activation(stM[:], ptm[:], AF.Copy)

        pc = pwork.tile([R, P], FP32, tag="pwork")
        nc.tensor.matmul(pc[:], convA[:, 0:R], stA[:],
                         start=True, stop=False, skip_group_check=True)
        nc.tensor.matmul(pc[:, 1:P], convA[:, R:2 * R], stA[:, 0:P - 1],
                         start=False, stop=False, skip_group_check=True)
        nc.tensor.matmul(pc[:, 0:P - 1], convA[0:3, 2 * R:3 * R],
                         stA[0:3, 1:P],
                         start=False, stop=False, skip_group_check=True)
        nc.tensor.matmul(pc[:], convM[:, 0:R], stM[:],
                         start=False, stop=False, skip_group_check=True)
        nc.tensor.matmul(pc[:, 1:P], convM[:, R:2 * R], stM[:, 0:P - 1],
                         start=False, stop=False, skip_group_check=True)
        nc.tensor.matmul(pc[:, 0:P - 1], convM[0:3, 2 * R:3 * R],
                         stM[0:3, 1:P],
                         start=False, stop=True, skip_group_check=True)
        sigb = spool.tile([R, P], FP32, tag="sigb")
        nc.scalar.activation(sigb[:], pc[:], AF.Sigmoid)
        psg = pwork.tile([P, R], FP32, tag="pwork")
        nc.tensor.transpose(psg[:], sigb[:], ident[0:R, 0:R])
        sig = apool.tile([P, R], FP32, tag="sig")
        nc.scalar.activation(sig[:], psg[:], AF.Copy)

        return ob, att, sig

    def finals(b, ob, att, sig):
        """ob_slice = (att + sig[:, r]) * ob_slice (= squares), then store."""
        for r in range(R):
            sl = ob[:, C * r:C * (r + 1)]
            if r % 8 >= POOL_R_CUT:
                # Pool has no TensorScalarPtr opcode -> two-op form
                tmp = tpool.tile([P, C], FP32, tag=f"ptmp{r % 2}")
                nc.gpsimd.tensor_scalar_add(tmp[:], att[:], sig[:, r:r + 1])
                nc.gpsimd.tensor_mul(sl, tmp[:], sl)
            else:
                nc.vector.scalar_tensor_tensor(sl, att[:], sig[:, r:r + 1],
                                               sl, op0=ALU.add, op1=ALU.mult)
        nc.sync.dma_start(
            out_d[b, :, :].rearrange("(p r) c -> p (r c)", p=P), ob[:])

    prev = None
    for b in range(NB):
        cur = (b, *compute_stats(b))
        if prev is not None:
            finals(*prev)
        prev = cur
    finals(*prev)


def _build_nc():
    nc = bacc.Bacc("TRN2", target_bir_lowering=False, debug=False,
                   enable_asserts=False, num_devices=N_CORES)
    x_d = nc.dram_tensor("xb", [NB, L, C], FP32, kind="ExternalInput").ap()
    w1_d = nc.dram_tensor("w1sb", [P, 2 * HB], FP32, kind="ExternalInput").ap()
    b1_d = nc.dram_tensor("b1col", [HB, 1], FP32, kind="ExternalInput").ap()
    w2b_d = nc.dram_tensor("w2b", [HB, C], FP32, kind="ExternalInput").ap()
    ca_d = nc.dram_tensor("convA", [R, 3 * R], FP32, kind="ExternalInput").ap()
    cm_d = nc.dram_tensor("convM", [R, 3 * R], FP32, kind="ExternalInput").ap()
    ones_d = nc.dram_tensor("ones17", [HB, P], FP32, kind="ExternalInput").ap()
    id_d = nc.dram_tensor("ident", [P, P], FP32, kind="ExternalInput").ap()
    rc_d = nc.dram_tensor("redcol", [P, 1], BF16, kind="ExternalInput").ap()
    out_d = nc.dram_tensor("out", [NB, L, C], FP32, kind="ExternalOutput").ap()

    with tile.TileContext(nc) as tc:
        with ExitStack() as ctx:
            _build_body(ctx, tc, out_d, x_d, w1_d, b1_d, w2b_d, ca_d, cm_d,
                        ones_d, id_d, rc_d)
    nc.compile()
    return nc


def get_nc():
    if "nc" not in _CACHE:
        _CACHE["nc"] = _build_nc()
    return _CACHE["nc"]


def _prep_inputs(W1, b1, W2, b2, conv_w):
    """Host-side parameter preprocessing (shared across cores)."""
    W1 = np.asarray(W1, np.float32)
    W2 = np.asarray(W2, np.float32)
    b1 = np.asarray(b1, np.float32)
    b2 = np.asarray(b2, np.float32)
    conv_w = np.asarray(conv_w, np.float32)

    w1sb = np.zeros((P, 2 * HB), np.float32)
    for h in range(2):
        w1sb[:, HB * h:HB * h + HID] = W1[P * h:P * (h + 1), :]
    w2b = np.concatenate([W2, b2[None, :]], axis=0).astype(np.float32)
    b1col = np.concatenate([b1, [1.0]]).astype(np.float32).reshape(HB, 1)

    # Banded Toeplitz over two adjacent 32-blocks; avg band folds in the
    # 1/C spatial-mean scale (device computes raw channel sums).
    def band64(w):
        Wb = np.zeros((64, 64), np.float32)
        for i in range(64):
            for k in range(7):
                j = i + k - 3
                if 0 <= j < 64:
                    Wb[i, j] = w[k]
        return Wb

    def pack(Wb):
        cv = np.zeros((R, 3 * R), np.float32)
        cv[:, 0:R] = Wb[0:R, 0:R].T          # main band
        cv[:, R:2 * R] = Wb[R:2 * R, 0:R].T  # prev-column corner
        cv[0:3, 2 * R:3 * R] = Wb[0:R, R:2 * R].T[0:3, :]  # next-column
        return cv

    convA = pack(band64(conv_w[:, 0, 0] / C))
    convM = pack(band64(conv_w[:, 1, 0]))

    return {
        "w1sb": w1sb,
        "b1col": np.ascontiguousarray(b1col),
        "w2b": w2b,
        "convA": np.ascontiguousarray(convA),
        "convM": np.ascontiguousarray(convM),
        "ones17": np.ones((HB, P), np.float32),
        "ident": np.eye(P, dtype=np.float32),
        "redcol": np.full((P, 1), 1.0 / L, ml_dtypes.bfloat16),
    }


def kernel(x, W1, b1, W2, b2, conv_w):
    nc = get_nc()
    x = np.asarray(x, np.float32)
    params = _prep_inputs(W1, b1, W2, b2, conv_w)
    in_maps = []
    for c in range(N_CORES):
        m = dict(params)
        m["xb"] = np.ascontiguousarray(x[NB * c:NB * (c + 1)])
        in_maps.append(m)
    _CACHE["last_in_maps"] = in_maps
    res = run_bass_kernel_spmd(nc, in_maps, list(range(N_CORES)))
    _CACHE["last_results"] = res
    return np.concatenate([res.results[c]["out"] for c in range(N_CORES)],
                          axis=0)



# revision 3
# speedup vs baseline: 1.9000x; 1.9000x over previous
"""CBAM kernel for Trainium2, 8-way batch-parallel SPMD — v3.

out = x^2 * (att_c[b,c] + sigmoid(conv(spatial_stats))[b,l]) per the CBAM
reference (out = x*ca + x*sa with ca = x*att_c, sa = x*sigmoid(conv)).

v3 layout (same as v2): per core 4 batches; batch x[4096, 256] lives in
SBUF as one [128, 8192] bf16 tensor with partition p = l // 32, free col
= 256*(l % 32) + c (so l = 32p + r).  Input loads with one SWDGE
cast-DMA (fp32 -> bf16) per batch; OUTPUT is stored bf16 (host upcasts),
halving store traffic.

v3 finals (the big change vs v2): instead of 32 small per-r-slice ops
per batch (which left GpSimd 63% busy at ~4 ns/elem), the combine is
three full-width ops per batch:
  ACT : F  = Copy(sig broadcast over c)          [128, 8192] bf16
  DVE : F += att (broadcast over r, mid-dim stride-0 keeps 2x bf16)
  DVE : out = (F bypass) * ob   — TensorScalarPtr hits 4x_2p bf16 mode
ob = x^2 comes from one big ACT Square (bf16 out).

Engine split per batch:
  PE   : channel-sum (1/L ones-column matmuls), MLP, stats transposes,
         conv over L as banded-Toeplitz matmuls on transposed stats
  ACT  : square, sigmoids, F sig-expand (broadcast copy), PSUM copies
  DVE  : spatial sum/max fold trees (bf16 2x), channel-max fold tree +
         cross-partition machinery, F att-add, final 4x stt multiply
  POOL : input cast-DMAs (SWDGE) only — no compute
"""

import numpy as np
from contextlib import ExitStack

import ml_dtypes

import concourse.bacc as bacc
import concourse.bass as bass
import concourse.tile as tile
import concourse.mybir as mybir
from concourse.bass_utils import run_bass_kernel_spmd

AF = mybir.ActivationFunctionType
ALU = mybir.AluOpType
AX = mybir.AxisListType
FP32 = mybir.dt.float32
BF16 = mybir.dt.bfloat16

N_CORES = 8
B_FULL = 32
NB = B_FULL // N_CORES  # batches per core = 4
L = 4096
C = 256
HID = 16
HB = HID + 1
P = 128
R = 32  # L-rows per partition (l = 32*p + r)

_CACHE: dict = {}


def _fold_tree(nc, pool, src_ap, nr, w0, dtype_hi, out_tile, name,
               op, f32_from):
    """Binary fold of [128, (nr, w0)] view down to [128, nr] into out_tile.

    Levels with width >= f32_from stay in dtype_hi (bf16, 2x DVE rate);
    below that, intermediates are fp32.  src_ap is the [P, nr*w0] AP.
    """
    cur = src_ap
    w = w0
    lvl = 0
    while w > 1:
        hw = w // 2
        if hw == 1:
            out = out_tile
        else:
            dt = dtype_hi if hw >= f32_from else FP32
            out = pool.tile([P, nr * hw], dt, tag=f"{name}{lvl}",
                            name=f"{name}{lvl}")[:]
        cv = cur.rearrange("p (r c) -> p r c", c=w)
        ov = out.rearrange("p (r c) -> p r c", c=hw)
        nc.vector.tensor_tensor(ov, cv[:, :, 0:hw], cv[:, :, hw:w], op)
        cur = out
        w = hw
        lvl += 1


def _build_body(ctx: ExitStack, tc, out_d, x_d, w1_d, b1_d, w2b_d, ca_d,
                cm_d, ones_d, id_d, rc_d):
    nc = tc.nc

    const = ctx.enter_context(tc.tile_pool(name="const", bufs=1))
    xpool = ctx.enter_context(tc.tile_pool(name="x", bufs=1))
    opool = ctx.enter_context(tc.tile_pool(name="outb", bufs=1))
    fbig = ctx.enter_context(tc.tile_pool(name="fbig", bufs=1))
    fpool = ctx.enter_context(tc.tile_pool(name="fold", bufs=1))
    spool = ctx.enter_context(tc.tile_pool(name="stats", bufs=2))
    apool = ctx.enter_context(tc.tile_pool(name="att", bufs=2))
    pacc = ctx.enter_context(tc.tile_pool(name="pacc", bufs=2, space="PSUM"))
    pwork = ctx.enter_context(tc.tile_pool(name="pwork", bufs=4, space="PSUM"))

    w1 = const.tile([P, 2 * HB], FP32)
    nc.sync.dma_start(w1[:], w1_d[:])
    b1 = const.tile([HB, 1], FP32)
    nc.sync.dma_start(b1[:], b1_d[:])
    w2b = const.tile([HB, C], FP32)
    nc.sync.dma_start(w2b[:], w2b_d[:])
    convA = const.tile([R, 3 * R], FP32)
    nc.sync.dma_start(convA[:], ca_d[:])
    convM = const.tile([R, 3 * R], FP32)
    nc.sync.dma_start(convM[:], cm_d[:])
    ones = const.tile([HB, P], FP32)
    nc.sync.dma_start(ones[:], ones_d[:])
    ident = const.tile([P, P], FP32)
    nc.sync.dma_start(ident[:], id_d[:])
    redcol = const.tile([P, 1], BF16)
    nc.sync.dma_start(redcol[:], rc_d[:])

    # ---- prefetch all four batches (SWDGE cast fp32 -> bf16) ----
    xb = []
    for b in range(NB):
        xt = xpool.tile([P, R * C], BF16, tag=f"x{b}", name=f"x{b}")
        nc.gpsimd.dma_start(
            xt[:], x_d[b, :, :].rearrange("(p r) c -> p (r c)", p=P))
        xb.append(xt)

    def compute_stats(b):
        """Square into ob (bf16); stats trees; att & sig (bf16)."""
        x = xb[b][:]
        ob = opool.tile([P, R * C], BF16, tag=f"ob{b % 2}", name=f"ob{b % 2}")

        # square (ACT) into the bf16 out buffer — one full-width op
        nc.scalar.activation(ob[:], x, AF.Square)

        # channel sum over l (PE): lhsT = 1/L column, accumulate 32 r-slices
        pcs = pacc.tile([1, C], FP32, tag="pcs")
        for r in range(R):
            nc.tensor.matmul(pcs[:], redcol[:], x[:, C * r:C * (r + 1)],
                             start=(r == 0), stop=(r == R - 1),
                             skip_group_check=True)

        # spatial sum/max over c per (p, r): binary fold trees (DVE)
        sum_s = spool.tile([P, R], FP32, tag="sum_s")
        with nc.allow_low_precision("bf16 upper fold levels; tol 2e-2"):
            _fold_tree(nc, fpool, x, R, C, BF16, sum_s[:], "sa",
                       ALU.add, f32_from=64)
        max_s = spool.tile([P, R], FP32, tag="max_s")
        _fold_tree(nc, fpool, x, R, C, BF16, max_s[:], "sm",
                   ALU.max, f32_from=2)

        # channel max over l: fold r within partitions, then cross-partition
        mb = fpool.tile([P, R * C // 2], BF16, tag="mb", name="mb")
        nc.vector.tensor_max(mb[:], x[:, 0:R * C // 2], x[:, R * C // 2:])
        w = R * C // 4
        while w > C:
            nc.vector.tensor_max(mb[:, 0:w], mb[:, 0:w], mb[:, w:2 * w])
            w //= 2
        mbf = spool.tile([P, C], FP32, tag="mbf")
        nc.vector.tensor_max(mbf[:], mb[:, 0:C], mb[:, C:2 * C])

        # cross-partition chan-max: 32x32 block transpose, in-block reduce,
        # quadrant gather (scalar-queue DMAs), fold, scatter into stats_cm
        bt = spool.tile([P, C], FP32, tag="bt")
        nc.vector.transpose(bt[:], mbf[:])
        red = spool.tile([P, 8], FP32, tag="red")
        nc.vector.tensor_reduce(red[:],
                                bt[:].rearrange("p (bj s) -> p bj s", s=32),
                                axis=AX.X, op=ALU.max)
        cm32 = spool.tile([32, 32], FP32, tag="cm32")
        for a in range(4):
            nc.scalar.dma_start(cm32[:, 8 * a:8 * (a + 1)],
                                red[32 * a:32 * (a + 1), :])
        cmf = spool.tile([32, 8], FP32, tag="cmf")
        nc.vector.tensor_reduce(cmf[:],
                                cm32[:].rearrange("r (a bj) -> r bj a", a=4),
                                axis=AX.X, op=ALU.max)

        stats_cm = spool.tile([P, 4], FP32, tag="stats_cm")
        avg_row = spool.tile([1, C], FP32, tag="avg_row")
        nc.scalar.activation(avg_row[:], pcs[:], AF.Copy)
        for h in range(2):
            nc.scalar.dma_start(stats_cm[:, 2 * h:2 * h + 1],
                                avg_row[0:1, P * h:P * (h + 1)])
        for bj in range(8):
            q = 32 * (bj % 4)
            nc.scalar.dma_start(stats_cm[q:q + 32, 2 * (bj // 4) + 1:
                                         2 * (bj // 4) + 2],
                                cmf[:, bj:bj + 1])

        # shared MLP -> att [128, 256] bf16 broadcast over partitions.
        # Row HID carries a constant 1 so w2b's b2 row contributes 2*b2.
        ph = pwork.tile([HB, 2], FP32, tag="pwork")
        nc.tensor.matmul(ph[:], w1[:, 0:HB], stats_cm[:, 0:2],
                         start=True, stop=False, skip_group_check=True)
        nc.tensor.matmul(ph[:], w1[:, HB:2 * HB], stats_cm[:, 2:4],
                         start=False, stop=True, skip_group_check=True)
        hsb = spool.tile([HB, 2], FP32, tag="hsb")
        nc.scalar.activation(hsb[:], ph[:], AF.Relu, bias=b1[:])
        h2 = spool.tile([HB, 1], FP32, tag="h2")
        nc.vector.tensor_add(h2[:], hsb[:, 0:1], hsb[:, 1:2])
        h2r = spool.tile([HB, P], FP32, tag="h2r")
        nc.vector.tensor_scalar_mul(h2r[:], ones[:], h2[:])
        po = pwork.tile([P, C], FP32, tag="pwork")
        nc.tensor.matmul(po[:], h2r[:], w2b[:], start=True, stop=True,
                         skip_group_check=True)
        att = apool.tile([P, C], BF16, tag="att")
        nc.scalar.activation(att[:], po[:], AF.Sigmoid)

        # conv over l: transpose stats to [32, 128] (l = 32*col + row),
        # banded-Toeplitz matmuls over the 32-row blocks with corner terms
        # into adjacent columns, sigmoid, transpose back.
        pta = pwork.tile([R, P], FP32, tag="pwork")
        nc.tensor.transpose(pta[:], sum_s[:], ident[:])
        stA = spool.tile([R, P], FP32, tag="stA")
        nc.scalar.activation(stA[:], pta[:], AF.Copy)
        ptm = pwork.tile([R, P], FP32, tag="pwork")
        nc.tensor.transpose(ptm[:], max_s[:], ident[:])
        stM = spool.tile([R, P], FP32, tag="stM")
        nc.scalar.activation(stM[:], ptm[:], AF.Copy)

        pc = pwork.tile([R, P], FP32, tag="pwork")
        nc.tensor.matmul(pc[:], convA[:, 0:R], stA[:],
                         start=True, stop=False, skip_group_check=True)
        nc.tensor.matmul(pc[:, 1:P], convA[:, R:2 * R], stA[:, 0:P - 1],
                         start=False, stop=False, skip_group_check=True)
        nc.tensor.matmul(pc[:, 0:P - 1], convA[0:3, 2 * R:3 * R],
                         stA[0:3, 1:P],
                         start=False, stop=False, skip_group_check=True)
        nc.tensor.matmul(pc[:], convM[:, 0:R], stM[:],
                         start=False, stop=False, skip_group_check=True)
        nc.tensor.matmul(pc[:, 1:P], convM[:, R:2 * R], stM[:, 0:P - 1],
                         start=False, stop=False, skip_group_check=True)
        nc.tensor.matmul(pc[:, 0:P - 1], convM[0:3, 2 * R:3 * R],
                         stM[0:3, 1:P],
                         start=False, stop=True, skip_group_check=True)
        sigb = spool.tile([R, P], FP32, tag="sigb")
        nc.scalar.activation(sigb[:], pc[:], AF.Sigmoid)
        psg = pwork.tile([P, R], FP32, tag="pwork")
        nc.tensor.transpose(psg[:], sigb[:], ident[0:R, 0:R])
        sig = apool.tile([P, R], BF16, tag="sig")
        nc.scalar.activation(sig[:], psg[:], AF.Copy)

        return ob, att, sig

    def finals(b, ob, att, sig):
        """F = sig⊕att (broadcast), out = F * ob (4x stt), store bf16."""
        F = fbig.tile([P, R * C], BF16, tag=f"F{b % 2}", name=f"F{b % 2}")
        Fv = F[:].rearrange("p (r c) -> p r c", c=C)
        # ACT: expand sig [P, R] over c via stride-0 broadcast read
        sig_b = sig[:].unsqueeze(2).broadcast_to([P, R, C])
        nc.scalar.activation(Fv, sig_b, AF.Copy)
        # DVE: F += att broadcast over r (mid-dim stride-0 keeps 2x bf16)
        att_b = att[:].unsqueeze(1).broadcast_to([P, R, C])
        with nc.allow_low_precision("bf16 attention factors; tol 2e-2"):
            nc.vector.tensor_tensor(Fv, Fv, att_b, ALU.add)
            # DVE: out = (F bypass) * ob — all-bf16 SBUF hits 4x_2p mode
            nc.vector.scalar_tensor_tensor(ob[:], F[:], 0.0, ob[:],
                                           op0=ALU.bypass, op1=ALU.mult)
        nc.sync.dma_start(
            out_d[b, :, :].rearrange("(p r) c -> p (r c)", p=P), ob[:])

    prev = None
    for b in range(NB):
        cur = (b, *compute_stats(b))
        if prev is not None:
            finals(*prev)
        prev = cur
    finals(*prev)


def _build_nc():
    nc = bacc.Bacc("TRN2", target_bir_lowering=False, debug=False,
                   enable_asserts=False, num_devices=N_CORES)
    x_d = nc.dram_tensor("xb", [NB, L, C], FP32, kind="ExternalInput").ap()
    w1_d = nc.dram_tensor("w1sb", [P, 2 * HB], FP32, kind="ExternalInput").ap()
    b1_d = nc.dram_tensor("b1col", [HB, 1], FP32, kind="ExternalInput").ap()
    w2b_d = nc.dram_tensor("w2b", [HB, C], FP32, kind="ExternalInput").ap()
    ca_d = nc.dram_tensor("convA", [R, 3 * R], FP32, kind="ExternalInput").ap()
    cm_d = nc.dram_tensor("convM", [R, 3 * R], FP32, kind="ExternalInput").ap()
    ones_d = nc.dram_tensor("ones17", [HB, P], FP32, kind="ExternalInput").ap()
    id_d = nc.dram_tensor("ident", [P, P], FP32, kind="ExternalInput").ap()
    rc_d = nc.dram_tensor("redcol", [P, 1], BF16, kind="ExternalInput").ap()
    out_d = nc.dram_tensor("out", [NB, L, C], BF16, kind="ExternalOutput").ap()

    with tile.TileContext(nc) as tc:
        with ExitStack() as ctx:
            _build_body(ctx, tc, out_d, x_d, w1_d, b1_d, w2b_d, ca_d, cm_d,
                        ones_d, id_d, rc_d)
    nc.compile()
    return nc


def get_nc():
    if "nc" not in _CACHE:
        _CACHE["nc"] = _build_nc()
    return _CACHE["nc"]


def _prep_inputs(W1, b1, W2, b2, conv_w):
    """Host-side parameter preprocessing (shared across cores)."""
    W1 = np.asarray(W1, np.float32)
    W2 = np.asarray(W2, np.float32)
    b1 = np.asarray(b1, np.float32)
    b2 = np.asarray(b2, np.float32)
    conv_w = np.asarray(conv_w, np.float32)

    w1sb = np.zeros((P, 2 * HB), np.float32)
    for h in range(2):
        w1sb[:, HB * h:HB * h + HID] = W1[P * h:P * (h + 1), :]
    w2b = np.concatenate([W2, b2[None, :]], axis=0).astype(np.float32)
    b1col = np.concatenate([b1, [1.0]]).astype(np.float32).reshape(HB, 1)

    # Banded Toeplitz over two adjacent 32-blocks; avg band folds in the
    # 1/C spatial-mean scale (device computes raw channel sums).
    def band64(w):
        Wb = np.zeros((64, 64), np.float32)
        for i in range(64):
            for k in range(7):
                j = i + k - 3
                if 0 <= j < 64:
                    Wb[i, j] = w[k]
        return Wb

    def pack(Wb):
        cv = np.zeros((R, 3 * R), np.float32)
        cv[:, 0:R] = Wb[0:R, 0:R].T          # main band
        cv[:, R:2 * R] = Wb[R:2 * R, 0:R].T  # prev-column corner
        cv[0:3, 2 * R:3 * R] = Wb[0:R, R:2 * R].T[0:3, :]  # next-column
        return cv

    convA = pack(band64(conv_w[:, 0, 0] / C))
    convM = pack(band64(conv_w[:, 1, 0]))

    return {
        "w1sb": w1sb,
        "b1col": np.ascontiguousarray(b1col),
        "w2b": w2b,
        "convA": np.ascontiguousarray(convA),
        "convM": np.ascontiguousarray(convM),
        "ones17": np.ones((HB, P), np.float32),
        "ident": np.eye(P, dtype=np.float32),
        "redcol": np.full((P, 1), 1.0 / L, ml_dtypes.bfloat16),
    }


def kernel(x, W1, b1, W2, b2, conv_w):
    nc = get_nc()
    x = np.asarray(x, np.float32)
    params = _prep_inputs(W1, b1, W2, b2, conv_w)
    in_maps = []
    for c in range(N_CORES):
        m = dict(params)
        m["xb"] = np.ascontiguousarray(x[NB * c:NB * (c + 1)])
        in_maps.append(m)
    _CACHE["last_in_maps"] = in_maps
    res = run_bass_kernel_spmd(nc, in_maps, list(range(N_CORES)))
    _CACHE["last_results"] = res
    return np.concatenate(
        [np.asarray(res.results[c]["out"]).astype(np.float32)
         for c in range(N_CORES)], axis=0)


# revision 4
# speedup vs baseline: 2.0083x; 1.0570x over previous
"""CBAM kernel for Trainium2, 8-way batch-parallel SPMD — v3.

out = x^2 * (att_c[b,c] + sigmoid(conv(spatial_stats))[b,l]) per the CBAM
reference (out = x*ca + x*sa with ca = x*att_c, sa = x*sigmoid(conv)).

v3 layout (same as v2): per core 4 batches; batch x[4096, 256] lives in
SBUF as one [128, 8192] bf16 tensor with partition p = l // 32, free col
= 256*(l % 32) + c (so l = 32p + r).  Input loads with one SWDGE
cast-DMA (fp32 -> bf16) per batch; OUTPUT is stored bf16 (host upcasts),
halving store traffic.

v3 finals (the big change vs v2): instead of 32 small per-r-slice ops
per batch (which left GpSimd 63% busy at ~4 ns/elem), the combine is
three full-width ops per batch:
  ACT : F  = Copy(sig broadcast over c)          [128, 8192] bf16
  DVE : F += att (broadcast over r, mid-dim stride-0 keeps 2x bf16)
  DVE : out = (F bypass) * ob   — TensorScalarPtr hits 4x_2p bf16 mode
ob = x^2 comes from one big ACT Square (bf16 out).

Engine split per batch:
  PE   : channel-sum (1/L ones-column matmuls), MLP, stats transposes,
         conv over L as banded-Toeplitz matmuls on transposed stats
  ACT  : square, sigmoids, F sig-expand (broadcast copy), PSUM copies
  DVE  : spatial sum/max fold trees (bf16 2x), channel-max fold tree +
         cross-partition machinery, F att-add, final 4x stt multiply
  POOL : input cast-DMAs (SWDGE) only — no compute
"""

import numpy as np
from contextlib import ExitStack

import ml_dtypes

import concourse.bacc as bacc
import concourse.bass as bass
import concourse.tile as tile
import concourse.mybir as mybir
from concourse.bass_utils import run_bass_kernel_spmd

AF = mybir.ActivationFunctionType
ALU = mybir.AluOpType
AX = mybir.AxisListType
FP32 = mybir.dt.float32
BF16 = mybir.dt.bfloat16

N_CORES = 8
B_FULL = 32
NB = B_FULL // N_CORES  # batches per core = 4
L = 4096
C = 256
HID = 16
HB = HID + 1
P = 128
R = 32  # L-rows per partition (l = 32*p + r)

_CACHE: dict = {}


def _fold_tree(nc, pool, src_ap, nr, w0, dtype_hi, out_tile, name,
               op, f32_from):
    """Binary fold of [128, (nr, w0)] view down to [128, nr] into out_tile.

    Levels with width >= f32_from stay in dtype_hi (bf16, 2x DVE rate);
    below that, intermediates are fp32.  src_ap is the [P, nr*w0] AP.
    """
    cur = src_ap
    w = w0
    lvl = 0
    while w > 1:
        hw = w // 2
        if hw == 1:
            out = out_tile
        else:
            dt = dtype_hi if hw >= f32_from else FP32
            out = pool.tile([P, nr * hw], dt, tag=f"{name}{lvl}",
                            name=f"{name}{lvl}")[:]
        cv = cur.rearrange("p (r c) -> p r c", c=w)
        ov = out.rearrange("p (r c) -> p r c", c=hw)
        nc.vector.tensor_tensor(ov, cv[:, :, 0:hw], cv[:, :, hw:w], op)
        cur = out
        w = hw
        lvl += 1


def _build_body(ctx: ExitStack, tc, out_d, x_d, w1_d, b1_d, w2b_d, ca_d,
                cm_d, ones_d, id_d, rc_d):
    nc = tc.nc

    const = ctx.enter_context(tc.tile_pool(name="const", bufs=1))
    xpool = ctx.enter_context(tc.tile_pool(name="x", bufs=1))
    opool = ctx.enter_context(tc.tile_pool(name="outb", bufs=1))
    fbig = ctx.enter_context(tc.tile_pool(name="fbig", bufs=1))
    fpool = ctx.enter_context(tc.tile_pool(name="fold", bufs=1))
    spool = ctx.enter_context(tc.tile_pool(name="stats", bufs=2))
    apool = ctx.enter_context(tc.tile_pool(name="att", bufs=2))
    pacc = ctx.enter_context(tc.tile_pool(name="pacc", bufs=2, space="PSUM"))
    pwork = ctx.enter_context(tc.tile_pool(name="pwork", bufs=4, space="PSUM"))

    w1 = const.tile([P, 2 * HB], FP32)
    nc.sync.dma_start(w1[:], w1_d[:])
    b1 = const.tile([HB, 1], FP32)
    nc.sync.dma_start(b1[:], b1_d[:])
    w2b = const.tile([HB, C], FP32)
    nc.sync.dma_start(w2b[:], w2b_d[:])
    convA = const.tile([R, 3 * R], FP32)
    nc.sync.dma_start(convA[:], ca_d[:])
    convM = const.tile([R, 3 * R], FP32)
    nc.sync.dma_start(convM[:], cm_d[:])
    ones = const.tile([HB, P], FP32)
    nc.sync.dma_start(ones[:], ones_d[:])
    ident = const.tile([P, P], FP32)
    nc.sync.dma_start(ident[:], id_d[:])
    redcol = const.tile([P, 1], BF16)
    nc.sync.dma_start(redcol[:], rc_d[:])

    # ---- prefetch all four batches (SWDGE cast fp32 -> bf16) ----
    xb = []
    for b in range(NB):
        xt = xpool.tile([P, R * C], BF16, tag=f"x{b}", name=f"x{b}")
        nc.gpsimd.dma_start(
            xt[:], x_d[b, :, :].rearrange("(p r) c -> p (r c)", p=P))
        xb.append(xt)

    def compute_stats(b):
        """Square into ob (bf16); stats trees; att & sig (bf16)."""
        x = xb[b][:]
        ob = opool.tile([P, R * C], BF16, tag=f"ob{b % 2}", name=f"ob{b % 2}")

        # square (ACT) into the bf16 out buffer — one full-width op
        nc.scalar.activation(ob[:], x, AF.Square)

        # channel sum over l (PE): lhsT = 1/L column, accumulate 32 r-slices
        pcs = pacc.tile([1, C], FP32, tag="pcs")
        for r in range(R):
            nc.tensor.matmul(pcs[:], redcol[:], x[:, C * r:C * (r + 1)],
                             start=(r == 0), stop=(r == R - 1),
                             skip_group_check=True)

        # spatial sum/max over c per (p, r): binary fold trees (DVE)
        sum_s = spool.tile([P, R], FP32, tag="sum_s")
        with nc.allow_low_precision("bf16 upper fold levels; tol 2e-2"):
            _fold_tree(nc, fpool, x, R, C, BF16, sum_s[:], "sa",
                       ALU.add, f32_from=64)
        max_s = spool.tile([P, R], FP32, tag="max_s")
        _fold_tree(nc, fpool, x, R, C, BF16, max_s[:], "sm",
                   ALU.max, f32_from=2)

        # channel max over l: fold r within partitions, then cross-partition
        mb = fpool.tile([P, R * C // 2], BF16, tag="mb", name="mb")
        nc.vector.tensor_max(mb[:], x[:, 0:R * C // 2], x[:, R * C // 2:])
        w = R * C // 4
        while w > C:
            nc.vector.tensor_max(mb[:, 0:w], mb[:, 0:w], mb[:, w:2 * w])
            w //= 2
        mbf = spool.tile([P, C], FP32, tag="mbf")
        nc.vector.tensor_max(mbf[:], mb[:, 0:C], mb[:, C:2 * C])

        # cross-partition chan-max: 32x32 block transpose, in-block reduce,
        # quadrant gather (scalar-queue DMAs), fold, scatter into stats_cm
        bt = spool.tile([P, C], FP32, tag="bt")
        nc.vector.transpose(bt[:], mbf[:])
        red = spool.tile([P, 8], FP32, tag="red")
        nc.vector.tensor_reduce(red[:],
                                bt[:].rearrange("p (bj s) -> p bj s", s=32),
                                axis=AX.X, op=ALU.max)
        cm32 = spool.tile([32, 32], FP32, tag="cm32")
        for a in range(4):
            nc.scalar.dma_start(cm32[:, 8 * a:8 * (a + 1)],
                                red[32 * a:32 * (a + 1), :])
        cmf = spool.tile([32, 8], FP32, tag="cmf")
        nc.vector.tensor_reduce(cmf[:],
                                cm32[:].rearrange("r (a bj) -> r bj a", a=4),
                                axis=AX.X, op=ALU.max)

        stats_cm = spool.tile([P, 4], FP32, tag="stats_cm")
        avg_row = spool.tile([1, C], FP32, tag="avg_row")
        nc.scalar.activation(avg_row[:], pcs[:], AF.Copy)
        for h in range(2):
            nc.scalar.dma_start(stats_cm[:, 2 * h:2 * h + 1],
                                avg_row[0:1, P * h:P * (h + 1)])
        for bj in range(8):
            q = 32 * (bj % 4)
            nc.scalar.dma_start(stats_cm[q:q + 32, 2 * (bj // 4) + 1:
                                         2 * (bj // 4) + 2],
                                cmf[:, bj:bj + 1])

        # shared MLP -> att [128, 256] bf16 broadcast over partitions.
        # Row HID carries a constant 1 so w2b's b2 row contributes 2*b2.
        ph = pwork.tile([HB, 2], FP32, tag="pwork")
        nc.tensor.matmul(ph[:], w1[:, 0:HB], stats_cm[:, 0:2],
                         start=True, stop=False, skip_group_check=True)
        nc.tensor.matmul(ph[:], w1[:, HB:2 * HB], stats_cm[:, 2:4],
                         start=False, stop=True, skip_group_check=True)
        hsb = spool.tile([HB, 2], FP32, tag="hsb")
        nc.scalar.activation(hsb[:], ph[:], AF.Relu, bias=b1[:])
        h2 = spool.tile([HB, 1], FP32, tag="h2")
        nc.vector.tensor_add(h2[:], hsb[:, 0:1], hsb[:, 1:2])
        h2r = spool.tile([HB, P], FP32, tag="h2r")
        nc.vector.tensor_scalar_mul(h2r[:], ones[:], h2[:])
        po = pwork.tile([P, C], FP32, tag="pwork")
        nc.tensor.matmul(po[:], h2r[:], w2b[:], start=True, stop=True,
                         skip_group_check=True)
        att = apool.tile([P, C], BF16, tag="att")
        nc.scalar.activation(att[:], po[:], AF.Sigmoid)

        # conv over l: transpose stats to [32, 128] (l = 32*col + row),
        # banded-Toeplitz matmuls over the 32-row blocks with corner terms
        # into adjacent columns, sigmoid, transpose back.
        pta = pwork.tile([R, P], FP32, tag="pwork")
        nc.tensor.transpose(pta[:], sum_s[:], ident[:])
        stA = spool.tile([R, P], FP32, tag="stA")
        nc.scalar.activation(stA[:], pta[:], AF.Copy)
        ptm = pwork.tile([R, P], FP32, tag="pwork")
        nc.tensor.transpose(ptm[:], max_s[:], ident[:])
        stM = spool.tile([R, P], FP32, tag="stM")
        nc.scalar.activation(stM[:], ptm[:], AF.Copy)

        pc = pwork.tile([R, P], FP32, tag="pwork")
        nc.tensor.matmul(pc[:], convA[:, 0:R], stA[:],
                         start=True, stop=False, skip_group_check=True)
        nc.tensor.matmul(pc[:, 1:P], convA[:, R:2 * R], stA[:, 0:P - 1],
                         start=False, stop=False, skip_group_check=True)
        nc.tensor.matmul(pc[:, 0:P - 1], convA[0:3, 2 * R:3 * R],
                         stA[0:3, 1:P],
                         start=False, stop=False, skip_group_check=True)
        nc.tensor.matmul(pc[:], convM[:, 0:R], stM[:],
                         start=False, stop=False, skip_group_check=True)
        nc.tensor.matmul(pc[:, 1:P], convM[:, R:2 * R], stM[:, 0:P - 1],
                         start=False, stop=False, skip_group_check=True)
        nc.tensor.matmul(pc[:, 0:P - 1], convM[0:3, 2 * R:3 * R],
                         stM[0:3, 1:P],
                         start=False, stop=True, skip_group_check=True)
        sigb = spool.tile([R, P], FP32, tag="sigb")
        nc.scalar.activation(sigb[:], pc[:], AF.Sigmoid)
        psg = pwork.tile([P, R], FP32, tag="pwork")
        nc.tensor.transpose(psg[:], sigb[:], ident[0:R, 0:R])
        sig = apool.tile([P, R], BF16, tag="sig")
        nc.scalar.activation(sig[:], psg[:], AF.Copy)

        return ob, att, sig

    def finals(b, ob, att, sig):
        """F = sig⊕att (broadcast), out = F * ob (4x stt), store bf16."""
        F = fbig.tile([P, R * C], BF16, tag=f"F{b % 2}", name=f"F{b % 2}")
        Fv = F[:].rearrange("p (r c) -> p r c", c=C)
        # ACT: expand sig [P, R] over c via stride-0 broadcast read
        sig_b = sig[:].unsqueeze(2).broadcast_to([P, R, C])
        nc.scalar.activation(Fv, sig_b, AF.Copy)
        # DVE: F += att broadcast over r (mid-dim stride-0 keeps 2x bf16)
        att_b = att[:].unsqueeze(1).broadcast_to([P, R, C])
        with nc.allow_low_precision("bf16 attention factors; tol 2e-2"):
            nc.vector.tensor_tensor(Fv, Fv, att_b, ALU.add)
            # DVE: out = F * ob — all-bf16 packed hits 2x_1p mode
            nc.vector.tensor_tensor(ob[:], F[:], ob[:], ALU.mult)
        nc.sync.dma_start(
            out_d[b, :, :].rearrange("(p r) c -> p (r c)", p=P), ob[:])

    prev = None
    for b in range(NB):
        cur = (b, *compute_stats(b))
        if prev is not None:
            finals(*prev)
        prev = cur
    finals(*prev)


def _build_nc():
    nc = bacc.Bacc("TRN2", target_bir_lowering=False, debug=False,
                   enable_asserts=False, num_devices=N_CORES)
    x_d = nc.dram_tensor("xb", [NB, L, C], FP32, kind="ExternalInput").ap()
    w1_d = nc.dram_tensor("w1sb", [P, 2 * HB], FP32, kind="ExternalInput").ap()
    b1_d = nc.dram_tensor("b1col", [HB, 1], FP32, kind="ExternalInput").ap()
    w2b_d = nc.dram_tensor("w2b", [HB, C], FP32, kind="ExternalInput").ap()
    ca_d = nc.dram_tensor("convA", [R, 3 * R], FP32, kind="ExternalInput").ap()
    cm_d = nc.dram_tensor("convM", [R, 3 * R], FP32, kind="ExternalInput").ap()
    ones_d = nc.dram_tensor("ones17", [HB, P], FP32, kind="ExternalInput").ap()
    id_d = nc.dram_tensor("ident", [P, P], FP32, kind="ExternalInput").ap()
    rc_d = nc.dram_tensor("redcol", [P, 1], BF16, kind="ExternalInput").ap()
    out_d = nc.dram_tensor("out", [NB, L, C], BF16, kind="ExternalOutput").ap()

    with tile.TileContext(nc) as tc:
        with ExitStack() as ctx:
            _build_body(ctx, tc, out_d, x_d, w1_d, b1_d, w2b_d, ca_d, cm_d,
                        ones_d, id_d, rc_d)
    nc.compile()
    return nc


def get_nc():
    if "nc" not in _CACHE:
        _CACHE["nc"] = _build_nc()
    return _CACHE["nc"]


def _prep_inputs(W1, b1, W2, b2, conv_w):
    """Host-side parameter preprocessing (shared across cores)."""
    W1 = np.asarray(W1, np.float32)
    W2 = np.asarray(W2, np.float32)
    b1 = np.asarray(b1, np.float32)
    b2 = np.asarray(b2, np.float32)
    conv_w = np.asarray(conv_w, np.float32)

    w1sb = np.zeros((P, 2 * HB), np.float32)
    for h in range(2):
        w1sb[:, HB * h:HB * h + HID] = W1[P * h:P * (h + 1), :]
    w2b = np.concatenate([W2, b2[None, :]], axis=0).astype(np.float32)
    b1col = np.concatenate([b1, [1.0]]).astype(np.float32).reshape(HB, 1)

    # Banded Toeplitz over two adjacent 32-blocks; avg band folds in the
    # 1/C spatial-mean scale (device computes raw channel sums).
    def band64(w):
        Wb = np.zeros((64, 64), np.float32)
        for i in range(64):
            for k in range(7):
                j = i + k - 3
                if 0 <= j < 64:
                    Wb[i, j] = w[k]
        return Wb

    def pack(Wb):
        cv = np.zeros((R, 3 * R), np.float32)
        cv[:, 0:R] = Wb[0:R, 0:R].T          # main band
        cv[:, R:2 * R] = Wb[R:2 * R, 0:R].T  # prev-column corner
        cv[0:3, 2 * R:3 * R] = Wb[0:R, R:2 * R].T[0:3, :]  # next-column
        return cv

    convA = pack(band64(conv_w[:, 0, 0] / C))
    convM = pack(band64(conv_w[:, 1, 0]))

    return {
        "w1sb": w1sb,
        "b1col": np.ascontiguousarray(b1col),
        "w2b": w2b,
        "convA": np.ascontiguousarray(convA),
        "convM": np.ascontiguousarray(convM),
        "ones17": np.ones((HB, P), np.float32),
        "ident": np.eye(P, dtype=np.float32),
        "redcol": np.full((P, 1), 1.0 / L, ml_dtypes.bfloat16),
    }


def kernel(x, W1, b1, W2, b2, conv_w):
    nc = get_nc()
    x = np.asarray(x, np.float32)
    params = _prep_inputs(W1, b1, W2, b2, conv_w)
    in_maps = []
    for c in range(N_CORES):
        m = dict(params)
        m["xb"] = np.ascontiguousarray(x[NB * c:NB * (c + 1)])
        in_maps.append(m)
    _CACHE["last_in_maps"] = in_maps
    res = run_bass_kernel_spmd(nc, in_maps, list(range(N_CORES)))
    _CACHE["last_results"] = res
    return np.concatenate(
        [np.asarray(res.results[c]["out"]).astype(np.float32)
         for c in range(N_CORES)], axis=0)


# revision 6
# speedup vs baseline: 2.1383x; 1.0647x over previous
"""CBAM kernel for Trainium2, 8-way batch-parallel SPMD — v4.

out = x^2 * (att_c[b,c] + sigmoid(conv(spatial_stats))[b,l]) per the CBAM
reference (out = x*ca + x*sa with ca = x*att_c, sa = x*sigmoid(conv)).

Layout: per core 4 batches; batch x[4096, 256] lives in SBUF as one
[128, 8192] bf16 tensor with partition p = l // 32, free col = 256*(l % 32)
+ c (so l = 32p + r).  Output stored bf16 (host upcasts).

v4 vs v3: the whole pipeline is half-split (r 0..15 / 16..31) for finer
overlap — input cast-DMAs, squares, spatial fold trees, F expand/add,
final multiply and output stores all run per half-batch.  Fold trees
stop at width 8 and finish with one tensor_reduce.  The sigmoid ACT
table set is preloaded at t=0 so no mid-kernel ACT_TABLE_LOAD.

Engine split per batch:
  PE   : channel-sum (1/L ones-column matmuls), MLP, stats transposes,
         conv over L as banded-Toeplitz matmuls on transposed stats
  ACT  : squares, sigmoids, F sig-expand (broadcast copy), PSUM copies
  DVE  : spatial sum/max fold trees (bf16 2x), channel-max fold tree +
         cross-partition machinery, F att-add (2x), final multiply (2x)
  POOL : input cast-DMAs (SWDGE) only — no compute
"""

import numpy as np
from contextlib import ExitStack

import ml_dtypes

import concourse.bacc as bacc
import concourse.bass as bass
import concourse.tile as tile
import concourse.mybir as mybir
from concourse.bass_utils import run_bass_kernel_spmd

AF = mybir.ActivationFunctionType
ALU = mybir.AluOpType
AX = mybir.AxisListType
FP32 = mybir.dt.float32
BF16 = mybir.dt.bfloat16

N_CORES = 8
B_FULL = 32
NB = B_FULL // N_CORES  # batches per core = 4
L = 4096
C = 256
HID = 16
HB = HID + 1
P = 128
R = 32   # L-rows per partition (l = 32*p + r)
RH = 16  # rows per half
HC = RH * C  # 4096 free cols per half

_CACHE: dict = {}


def _fold_tree(nc, pool, src_ap, nr, w0, out_ap, name, op, f32_from,
               red_from=8):
    """Fold [128, (nr, w0)] down to [128, nr] into out_ap.

    Binary tt folds until width red_from, then one tensor_reduce.
    Levels with width >= f32_from stay bf16 (2x DVE rate).
    """
    cur = src_ap
    w = w0
    lvl = 0
    while w > red_from:
        hw = w // 2
        dt = BF16 if hw >= f32_from else FP32
        out = pool.tile([P, nr * hw], dt, tag=f"{name}{lvl}",
                        name=f"{name}{lvl}")[:]
        cv = cur.rearrange("p (r c) -> p r c", c=w)
        ov = out.rearrange("p (r c) -> p r c", c=hw)
        nc.vector.tensor_tensor(ov, cv[:, :, 0:hw], cv[:, :, hw:w], op)
        cur = out
        w = hw
        lvl += 1
    nc.vector.tensor_reduce(out_ap,
                            cur.rearrange("p (r c) -> p r c", c=w),
                            axis=AX.X, op=op)


def _build_body(ctx: ExitStack, tc, out_d, x_d, w1_d, b1_d, w2b_d, ca_d,
                cm_d, ones_d, id_d, rc_d):
    nc = tc.nc

    const = ctx.enter_context(tc.tile_pool(name="const", bufs=1))
    xpool = ctx.enter_context(tc.tile_pool(name="x", bufs=1))
    opool = ctx.enter_context(tc.tile_pool(name="outb", bufs=1))
    fbig = ctx.enter_context(tc.tile_pool(name="fbig", bufs=1))
    fpool = ctx.enter_context(tc.tile_pool(name="fold", bufs=1))
    spool = ctx.enter_context(tc.tile_pool(name="stats", bufs=2))
    apool = ctx.enter_context(tc.tile_pool(name="att", bufs=2))
    pacc = ctx.enter_context(tc.tile_pool(name="pacc", bufs=2, space="PSUM"))
    pwork = ctx.enter_context(tc.tile_pool(name="pwork", bufs=4, space="PSUM"))

    w1 = const.tile([P, 2 * HB], FP32)
    nc.sync.dma_start(w1[:], w1_d[:])
    b1 = const.tile([HB, 1], FP32)
    nc.sync.dma_start(b1[:], b1_d[:])
    w2b = const.tile([HB, C], FP32)
    nc.sync.dma_start(w2b[:], w2b_d[:])
    convA = const.tile([R, 3 * R], FP32)
    nc.sync.dma_start(convA[:], ca_d[:])
    convM = const.tile([R, 3 * R], FP32)
    nc.sync.dma_start(convM[:], cm_d[:])
    ones = const.tile([HB, P], FP32)
    nc.sync.dma_start(ones[:], ones_d[:])
    ident = const.tile([P, P], FP32)
    nc.sync.dma_start(ident[:], id_d[:])
    redcol = const.tile([P, 1], BF16)
    nc.sync.dma_start(redcol[:], rc_d[:])

    # preload the sigmoid ACT table set (contains Square/Copy/Relu too) so
    # no mid-kernel ACT_TABLE_LOAD lands on the critical path
    warm = spool.tile([1, 1], FP32, tag="warm")
    nc.scalar.activation(warm[:], ident[0:1, 0:1], AF.Sigmoid)

    # ---- prefetch all 8 half-batches (SWDGE cast fp32 -> bf16) ----
    xb = []
    for b in range(NB):
        xt = xpool.tile([P, R * C], BF16, tag=f"x{b}", name=f"x{b}")
        xd = x_d[b, :, :].rearrange("(p r) c -> p (r c)", p=P)
        for h in range(2):
            nc.gpsimd.dma_start(xt[:, HC * h:HC * (h + 1)],
                                xd[:, HC * h:HC * (h + 1)])
        xb.append(xt)

    def stats_half(b, h, pcs, sum_s, max_s):
        """Per-half: chan-sum matmuls, spatial sum/max trees."""
        x = xb[b][:]
        xh = x[:, HC * h:HC * (h + 1)]

        for r in range(RH):
            rr = RH * h + r
            nc.tensor.matmul(pcs[:], redcol[:], x[:, C * rr:C * (rr + 1)],
                             start=(rr == 0), stop=(rr == R - 1),
                             skip_group_check=True)

        with nc.allow_low_precision("bf16 fold levels; tol 2e-2"):
            _fold_tree(nc, fpool, xh, RH, C,
                       sum_s[:, RH * h:RH * (h + 1)], f"sa{h}",
                       ALU.add, f32_from=8)
        _fold_tree(nc, fpool, xh, RH, C,
                   max_s[:, RH * h:RH * (h + 1)], f"sm{h}",
                   ALU.max, f32_from=2)

    def stats_join(b, pcs, sum_s, max_s):
        """Chan-max fold + cross-partition, MLP -> att, conv -> sig."""
        x = xb[b][:]

        # channel max over l: fold r within partitions (first op folds the
        # two halves), then cross-partition machinery
        mb = fpool.tile([P, R * C // 2], BF16, tag="mb", name="mb")
        nc.vector.tensor_max(mb[:], x[:, 0:R * C // 2], x[:, R * C // 2:])
        w = R * C // 4
        while w > C:
            nc.vector.tensor_max(mb[:, 0:w], mb[:, 0:w], mb[:, w:2 * w])
            w //= 2
        mbf = spool.tile([P, C], FP32, tag="mbf")
        nc.vector.tensor_max(mbf[:], mb[:, 0:C], mb[:, C:2 * C])

        # cross-partition chan-max: 32x32 block transpose, in-block reduce,
        # quadrant gather (scalar-queue DMAs), fold, scatter into stats_cm
        bt = spool.tile([P, C], FP32, tag="bt")
        nc.vector.transpose(bt[:], mbf[:])
        red = spool.tile([P, 8], FP32, tag="red")
        nc.vector.tensor_reduce(red[:],
                                bt[:].rearrange("p (bj s) -> p bj s", s=32),
                                axis=AX.X, op=ALU.max)
        cm32 = spool.tile([32, 32], FP32, tag="cm32")
        for a in range(4):
            nc.scalar.dma_start(cm32[:, 8 * a:8 * (a + 1)],
                                red[32 * a:32 * (a + 1), :])
        cmf = spool.tile([32, 8], FP32, tag="cmf")
        nc.vector.tensor_reduce(cmf[:],
                                cm32[:].rearrange("r (a bj) -> r bj a", a=4),
                                axis=AX.X, op=ALU.max)

        stats_cm = spool.tile([P, 4], FP32, tag="stats_cm")
        avg_row = spool.tile([1, C], FP32, tag="avg_row")
        nc.scalar.activation(avg_row[:], pcs[:], AF.Copy)
        for hh in range(2):
            nc.scalar.dma_start(stats_cm[:, 2 * hh:2 * hh + 1],
                                avg_row[0:1, P * hh:P * (hh + 1)])
        for bj in range(8):
            q = 32 * (bj % 4)
            nc.scalar.dma_start(stats_cm[q:q + 32, 2 * (bj // 4) + 1:
                                         2 * (bj // 4) + 2],
                                cmf[:, bj:bj + 1])

        # shared MLP -> att [128, 256] bf16 broadcast over partitions.
        # Row HID carries a constant 1 so w2b's b2 row contributes 2*b2.
        ph = pwork.tile([HB, 2], FP32, tag="pwork")
        nc.tensor.matmul(ph[:], w1[:, 0:HB], stats_cm[:, 0:2],
                         start=True, stop=False, skip_group_check=True)
        nc.tensor.matmul(ph[:], w1[:, HB:2 * HB], stats_cm[:, 2:4],
                         start=False, stop=True, skip_group_check=True)
        hsb = spool.tile([HB, 2], FP32, tag="hsb")
        nc.scalar.activation(hsb[:], ph[:], AF.Relu, bias=b1[:])
        h2 = spool.tile([HB, 1], FP32, tag="h2")
        nc.vector.tensor_add(h2[:], hsb[:, 0:1], hsb[:, 1:2])
        h2r = spool.tile([HB, P], FP32, tag="h2r")
        nc.vector.tensor_scalar_mul(h2r[:], ones[:], h2[:])
        po = pwork.tile([P, C], FP32, tag="pwork")
        nc.tensor.matmul(po[:], h2r[:], w2b[:], start=True, stop=True,
                         skip_group_check=True)
        att = apool.tile([P, C], BF16, tag="att")
        nc.scalar.activation(att[:], po[:], AF.Sigmoid)

        # conv over l: transpose both stats into one PSUM tile [32, 256]
        # (l = 32*col + row per 128-col block), one ACT copy, then
        # banded-Toeplitz matmuls with corner terms, sigmoid, transpose back.
        ptam = pwork.tile([R, 2 * P], FP32, tag="pwork")
        nc.tensor.transpose(ptam[:, 0:P], sum_s[:], ident[:])
        nc.tensor.transpose(ptam[:, P:2 * P], max_s[:], ident[:])
        stAM = spool.tile([R, 2 * P], FP32, tag="stAM")
        nc.scalar.activation(stAM[:], ptam[:], AF.Copy)
        stA = stAM[:, 0:P]
        stM = stAM[:, P:2 * P]

        pc = pwork.tile([R, P], FP32, tag="pwork")
        nc.tensor.matmul(pc[:], convA[:, 0:R], stA,
                         start=True, stop=False, skip_group_check=True)
        nc.tensor.matmul(pc[:, 1:P], convA[:, R:2 * R], stA[:, 0:P - 1],
                         start=False, stop=False, skip_group_check=True)
        nc.tensor.matmul(pc[:, 0:P - 1], convA[0:3, 2 * R:3 * R],
                         stA[0:3, 1:P],
                         start=False, stop=False, skip_group_check=True)
        nc.tensor.matmul(pc[:], convM[:, 0:R], stM,
                         start=False, stop=False, skip_group_check=True)
        nc.tensor.matmul(pc[:, 1:P], convM[:, R:2 * R], stM[:, 0:P - 1],
                         start=False, stop=False, skip_group_check=True)
        nc.tensor.matmul(pc[:, 0:P - 1], convM[0:3, 2 * R:3 * R],
                         stM[0:3, 1:P],
                         start=False, stop=True, skip_group_check=True)
        sigb = spool.tile([R, P], FP32, tag="sigb")
        nc.scalar.activation(sigb[:], pc[:], AF.Sigmoid)
        psg = pwork.tile([P, R], FP32, tag="pwork")
        nc.tensor.transpose(psg[:], sigb[:], ident[0:R, 0:R])
        sig = apool.tile([P, R], BF16, tag="sig")
        nc.scalar.activation(sig[:], psg[:], AF.Copy)

        return att, sig

    def square_half(b, ob, h):
        x = xb[b][:]
        nc.scalar.activation(ob[:, HC * h:HC * (h + 1)],
                             x[:, HC * h:HC * (h + 1)], AF.Square)

    def finals_half(b, ob, att, sig, F, h):
        """F_h = sig⊕att (broadcasts), out_h = F_h * ob_h, store bf16."""
        Fv = F[:].rearrange("p (r c) -> p r c", c=C)[:, RH * h:RH * (h + 1)]
        sig_b = sig[:, RH * h:RH * (h + 1)].unsqueeze(2) \
                                           .broadcast_to([P, RH, C])
        nc.scalar.activation(Fv, sig_b, AF.Copy)
        att_b = att[:].unsqueeze(1).broadcast_to([P, RH, C])
        obh = ob[:, HC * h:HC * (h + 1)]
        with nc.allow_low_precision("bf16 attention factors; tol 2e-2"):
            nc.vector.tensor_tensor(Fv, Fv, att_b, ALU.add)
            nc.vector.tensor_tensor(obh, F[:, HC * h:HC * (h + 1)], obh,
                                    ALU.mult)
        nc.sync.dma_start(
            out_d[b, :, :].rearrange("(p r) c -> p (r c)",
                                     p=P)[:, HC * h:HC * (h + 1)], obh)

    def compute_stats(b):
        pcs = pacc.tile([1, C], FP32, tag="pcs")
        sum_s = spool.tile([P, R], FP32, tag="sum_s")
        max_s = spool.tile([P, R], FP32, tag="max_s")
        ob = opool.tile([P, R * C], BF16, tag=f"ob{b % 2}", name=f"ob{b % 2}")
        for h in range(2):
            stats_half(b, h, pcs, sum_s, max_s)
        att, sig = stats_join(b, pcs, sum_s, max_s)
        for h in range(2):
            square_half(b, ob, h)
        return ob, att, sig

    def finals(b, ob, att, sig):
        F = fbig.tile([P, R * C], BF16, tag=f"F{b % 2}", name=f"F{b % 2}")
        for h in range(2):
            finals_half(b, ob, att, sig, F, h)

    prev = None
    for b in range(NB):
        cur = (b, *compute_stats(b))
        if prev is not None:
            finals(*prev)
        prev = cur
    finals(*prev)


def _build_nc():
    nc = bacc.Bacc("TRN2", target_bir_lowering=False, debug=False,
                   enable_asserts=False, num_devices=N_CORES)
    x_d = nc.dram_tensor("xb", [NB, L, C], FP32, kind="ExternalInput").ap()
    w1_d = nc.dram_tensor("w1sb", [P, 2 * HB], FP32, kind="ExternalInput").ap()
    b1_d = nc.dram_tensor("b1col", [HB, 1], FP32, kind="ExternalInput").ap()
    w2b_d = nc.dram_tensor("w2b", [HB, C], FP32, kind="ExternalInput").ap()
    ca_d = nc.dram_tensor("convA", [R, 3 * R], FP32, kind="ExternalInput").ap()
    cm_d = nc.dram_tensor("convM", [R, 3 * R], FP32, kind="ExternalInput").ap()
    ones_d = nc.dram_tensor("ones17", [HB, P], FP32, kind="ExternalInput").ap()
    id_d = nc.dram_tensor("ident", [P, P], FP32, kind="ExternalInput").ap()
    rc_d = nc.dram_tensor("redcol", [P, 1], BF16, kind="ExternalInput").ap()
    out_d = nc.dram_tensor("out", [NB, L, C], BF16, kind="ExternalOutput").ap()

    with tile.TileContext(nc) as tc:
        with ExitStack() as ctx:
            _build_body(ctx, tc, out_d, x_d, w1_d, b1_d, w2b_d, ca_d, cm_d,
                        ones_d, id_d, rc_d)
    nc.compile()
    return nc


def get_nc():
    if "nc" not in _CACHE:
        _CACHE["nc"] = _build_nc()
    return _CACHE["nc"]


def _prep_inputs(W1, b1, W2, b2, conv_w):
    """Host-side parameter preprocessing (shared across cores)."""
    W1 = np.asarray(W1, np.float32)
    W2 = np.asarray(W2, np.float32)
    b1 = np.asarray(b1, np.float32)
    b2 = np.asarray(b2, np.float32)
    conv_w = np.asarray(conv_w, np.float32)

    w1sb = np.zeros((P, 2 * HB), np.float32)
    for h in range(2):
        w1sb[:, HB * h:HB * h + HID] = W1[P * h:P * (h + 1), :]
    w2b = np.concatenate([W2, b2[None, :]], axis=0).astype(np.float32)
    b1col = np.concatenate([b1, [1.0]]).astype(np.float32).reshape(HB, 1)

    # Banded Toeplitz over two adjacent 32-blocks; avg band folds in the
    # 1/C spatial-mean scale (device computes raw channel sums).
    def band64(w):
        Wb = np.zeros((64, 64), np.float32)
        for i in range(64):
            for k in range(7):
                j = i + k - 3
                if 0 <= j < 64:
                    Wb[i, j] = w[k]
        return Wb

    def pack(Wb):
        cv = np.zeros((R, 3 * R), np.float32)
        cv[:, 0:R] = Wb[0:R, 0:R].T          # main band
        cv[:, R:2 * R] = Wb[R:2 * R, 0:R].T  # prev-column corner
        cv[0:3, 2 * R:3 * R] = Wb[0:R, R:2 * R].T[0:3, :]  # next-column
        return cv

    convA = pack(band64(conv_w[:, 0, 0] / C))
    convM = pack(band64(conv_w[:, 1, 0]))

    return {
        "w1sb": w1sb,
        "b1col": np.ascontiguousarray(b1col),
        "w2b": w2b,
        "convA": np.ascontiguousarray(convA),
        "convM": np.ascontiguousarray(convM),
        "ones17": np.ones((HB, P), np.float32),
        "ident": np.eye(P, dtype=np.float32),
        "redcol": np.full((P, 1), 1.0 / L, ml_dtypes.bfloat16),
    }


def kernel(x, W1, b1, W2, b2, conv_w):
    nc = get_nc()
    x = np.asarray(x, np.float32)
    params = _prep_inputs(W1, b1, W2, b2, conv_w)
    in_maps = []
    for c in range(N_CORES):
        m = dict(params)
        m["xb"] = np.ascontiguousarray(x[NB * c:NB * (c + 1)])
        in_maps.append(m)
    _CACHE["last_in_maps"] = in_maps
    res = run_bass_kernel_spmd(nc, in_maps, list(range(N_CORES)))
    _CACHE["last_results"] = res
    return np.concatenate(
        [np.asarray(res.results[c]["out"]).astype(np.float32)
         for c in range(N_CORES)], axis=0)
